# revision 47
# baseline (speedup 1.0000x reference)
"""Trainium2 Bass kernel for nn_Attention_50637664420407.

Dense causal transformer block: LayerNorm -> QKV -> RoPE -> causal attention
-> out-projection.  x:[2,2048,1024] f32.

Sharding (8 cores): head-parallel.  Core c owns heads {2c, 2c+1} for both
batch elements.  W_qkv is split column-wise per head group, W_out row-wise;
each core computes a full [4096,1024] partial of the output projection (bf16)
and the host sums the 8 partials.

Layout: feature-major ("transposed") for x/q/k; V is produced token-major
directly from the QKV matmul (lhsT = x chunk), which kills the per-pair PE
transposes.  LayerNorm is folded algebraically into the QKV matmul:
    qkv[t,c] = rstd_t * (P[c,t] - mu_t*G[c] + bq[c]*riv_t)
with P = Wg^T x^T (Wg = ln_g-scaled W), G = colsum(Wg), bq = ln_b @ W,
riv = 1/rstd.  The q-side rstd is applied at eviction via a Pool-engine
partition_broadcast of the rstd row; the K-SIDE rstd IS FOLDED INTO THE
SOFTMAX EXP's per-partition scale (scoresT has k-tokens on partitions), so
kn is stored unscaled.  V's rstd is a per-partition scalar in token-major
layout (rstd transposed via tiny PE transposes).

Attention computes scores TRANSPOSED (scoresT[j,i]); softmax denominator is
the 65th row of the AV matmul (ones column in the token-major V tile); the
reciprocal runs on DVE ([1,512] per 512 tokens) and is broadcast across 64
partitions by a tiny ones-matmul into the free rows 64:128 of the same AV
PSUM tile, so normalization costs one tensor_mul at eviction.

Everything on the PE is bf16 (PSUM accumulation stays f32); rel-err budget
2e-2 vs observed ~2e-3.  All DMAs are batched (one per x tile / weight) and
there are no DRAM round-trips.
"""

import sys
import numpy as np

for _p in ("/opt/trn_rl_repo", "/root/.axon_site/_ro/trn_rl_repo"):
    if _p not in sys.path:
        sys.path.append(_p)

import ml_dtypes
import concourse.bass as bass
import concourse.bacc as bacc
import concourse.mybir as mybir
import concourse.tile as tile
from concourse.alu_op_type import AluOpType

F32 = mybir.dt.float32
F32R = mybir.dt.float32r
BF16 = mybir.dt.bfloat16
AF = mybir.ActivationFunctionType
BF = ml_dtypes.bfloat16

P = 128          # partitions
T = 4096         # total tokens (2 batches x 2048)
NT = 2048        # seq len per batch
DIMK = 1024      # model dim
KC = 8           # k chunks of 128
TT = 8           # token tiles of 512
D = 64           # head dim
SCALE = D ** -0.5  # 0.125


def ts(i, n):
    return slice(i * n, (i + 1) * n)


def r32(ap):
    return ap.bitcast(F32R)


class _Bacc(bacc.Bacc):
    """Bacc with a pinned ACT table-set choice (see baseline docstring):
    restrict Exp/Ln/Square to natural_log_exp_and_others so the whole kernel
    needs a single table load."""

    def insert_act_table_loads(self):
        import concourse.bass_isa as bass_isa  # noqa: F401
        from concourse.hw_specs import get_activation_tables
        import bass_rust as _bass_rust
        has_activation = any(
            isinstance(i, mybir.InstActivation)
            for b in self.main_func.blocks
            for i in b.instructions
        )
        if not has_activation:
            return
        pinned = {AF.Exp, AF.Ln, AF.Square}
        keep = "natural_log_exp_and_others"
        tables = []
        for name, funcs in get_activation_tables(self.m.arch).items():
            if name != keep:
                funcs = funcs - pinned
            tables.append((name, funcs))
        _bass_rust.insert_act_table_loads(self, tables)


def build_program():
    nc = _Bacc("TRN2", target_bir_lowering=False, debug=False)

    xt_h = nc.declare_dram_parameter("xt", [DIMK, T], BF16, False)
    wq_h = nc.declare_dram_parameter("wqkv", [DIMK, 384], BF16, False)
    wo_h = nc.declare_dram_parameter("wo", [P, DIMK], BF16, False)
    gb_h = nc.declare_dram_parameter("gb", [3, 384], BF16, False)  # [0; -G; bq]
    cc_h = nc.declare_dram_parameter("cc", [P, NT], BF16, False)  # cos, 2-head tiled
    ss_h = nc.declare_dram_parameter("ss", [P, NT], BF16, False)  # signed sin
    tri_h = nc.declare_dram_parameter("tri", [P, P], BF16, False)  # tri[j,i] = i>=j
    perm_h = nc.declare_dram_parameter("perm", [P, P], BF16, False)  # rotate-half
    id1_h = nc.declare_dram_parameter("id1", [1, 16], F32, False)   # 1x1.. identity row
    idn_h = nc.declare_dram_parameter("idn", [P, P], F32, False)    # 128x128 identity
    onesb_h = nc.declare_dram_parameter("onesb", [P, 64], BF16, False)  # vtok ones cols
    onesc_h = nc.declare_dram_parameter("onesc", [P, 16], BF16, False)  # ones lhsT col
    onesr_h = nc.declare_dram_parameter("onesr", [1, P], F32R, False)   # ones row (f32r)
    out_h = nc.declare_dram_parameter("out", [T, DIMK], BF16, True)

    with tile.TileContext(nc) as tc:
        with tc.tile_pool(name="const", bufs=1) as const, \
             tc.tile_pool(name="qkvsb", bufs=1) as qkvsb, \
             tc.tile_pool(name="ohp", bufs=1) as ohp:

            # ---------- persistent sbuf tensors ----------
            onesc_t = const.tile([P, 16], BF16)
            nc.sync.dma_start(out=onesc_t, in_=onesc_h[:, :])
            ones_t = onesc_t[:, 0:1]          # [128,1] ones (bf16 lhsT)
            onesr_t = const.tile([1, P], F32R)  # [1,128] ones row (f32r lhsT)
            xk0 = const.tile([P, KC, 512], BF16)
            for _half in range(2):
                nc.sync.dma_start(
                    out=xk0[:, ts(_half, 4), :],
                    in_=xt_h[ts(_half, 4 * P), ts(0, 512)].rearrange(
                        "(k p) t -> p k t", p=P))
            id1_t = const.tile([1, 16], F32)  # identity for [1,128] transposes
            nc.sync.dma_start(out=id1_t, in_=id1_h[:, :])
            idn_t = const.tile([P, P], F32)   # identity for [128,1] transposes
            nc.sync.dma_start(out=idn_t, in_=idn_h[:, :])
            w_t = const.tile([P, KC, 384], BF16)   # W_qkv chunks -> ln_g-scaled
            nc.sync.dma_start(out=w_t, in_=wq_h[:, :].rearrange("(k p) c -> p k c", p=P))
            tri_t = const.tile([P, P], BF16)
            wo_t = const.tile([P, DIMK], BF16)
            gb_t = const.tile([3, 384], BF16)      # [0; -G; bq]
            nc.sync.dma_start(out=gb_t, in_=gb_h[:, :])
            eps1 = const.tile([1, 1], F32)
            nc.vector.memset(eps1, 1e-5)
            eps128 = const.tile([P, 1], F32)
            nc.vector.memset(eps128, 1e-5)
            rsT_t = const.tile([P, 32], F32)       # rstd, token-major by 128-block
            scT_t = const.tile([P, 32], F32)       # SCALE*rstd, token-major

            # q/k feature-major bf16; v token-major bf16 with ones cols at 64/129
            qn = qkvsb.tile([P, T], BF16)
            kn = qkvsb.tile([P, T], BF16)
            vtok = qkvsb.tile([P, 32, 130], BF16)  # [t128, block, head*65+d]
            ohT = ohp.tile([P, T], BF16)           # attention output, feature-major

            # ---------- phases A-C: stats + QKV + RoPE, per 512-token tile ----
            with tc.tile_pool(name="statsb", bufs=3) as statsb, \
                 tc.tile_pool(name="stg", bufs=2) as stg, \
                 tc.tile_pool(name="xtc", bufs=8) as xtc, \
                 tc.tile_pool(name="xsq", bufs=3) as xsqp, \
                 tc.tile_pool(name="rbp", bufs=2) as rbp, \
                 tc.tile_pool(name="qkps", bufs=2, space="PSUM") as qkps, \
                 tc.tile_pool(name="vps", bufs=1, space="PSUM") as vps, \
                 tc.tile_pool(name="stps", bufs=1, space="PSUM") as stps, \
                 tc.tile_pool(name="rpps", bufs=1, space="PSUM") as rpps:

                perm_t = statsb.tile([P, P], BF16, name="permt")
                nc.sync.dma_start(out=perm_t, in_=perm_h[:, :])
                cc_t = statsb.tile([P, NT], BF16, name="cct")
                ss_t = statsb.tile([P, NT], BF16, name="sst")
                nc.sync.dma_start(out=cc_t, in_=cc_h[:, :])
                nc.sync.dma_start(out=ss_t, in_=ss_h[:, :])

                def emit_tile_stats(t):
                    if t == 0:
                        xk = xk0
                    else:
                        xk = xtc.tile([P, KC, 512], BF16, tag="x")
                        for half in range(2):
                            nc.sync.dma_start(
                                out=xk[:, ts(half, 4), :],
                                in_=xt_h[ts(half, 4 * P), ts(t, 512)].rearrange(
                                    "(k p) t -> p k t", p=P))
                    # squares first (Pool is slowest; give every engine a head start)
                    sqs = {}
                    for k in (1, 2, 5, 6, 7, 0, 3, 4):
                        sq = xsqp.tile([P, 512], BF16, tag=f"sq{k}")
                        sqs[k] = sq
                        if k in (3, 4):
                            nc.gpsimd.tensor_mul(sq, xk[:, k, :], xk[:, k, :])
                        elif k in (1, 2, 5):
                            nc.vector.tensor_mul(sq, xk[:, k, :], xk[:, k, :])
                        else:
                            nc.scalar.activation(out=sq, in_=xk[:, k, :], func=AF.Square)
                    # token-major stats: S1T/S2T via free-1 matmuls (out [128,1])
                    st_ps = stps.tile([P, 8], F32, tag="s12")
                    for blk in range(4):
                        for k in range(KC):
                            nc.tensor.matmul(st_ps[:, blk : blk + 1],
                                             lhsT=xk[:, k, ts(blk, P)], rhs=ones_t,
                                             start=(k == 0), stop=(k == KC - 1),
                                             skip_group_check=True)
                    korder = (1, 2, 5, 0, 6, 7, 3, 4)
                    for blk in range(4):
                        for i, k in enumerate(korder):
                            nc.tensor.matmul(st_ps[:, 4 + blk : 5 + blk],
                                             lhsT=sqs[k][:, ts(blk, P)], rhs=ones_t,
                                             start=(i == 0), stop=(i == KC - 1),
                                             skip_group_check=True)
                    # token-major stats math on [128,4] (free 4 -> ~free);
                    # mu and riv land interleaved in mr so ONE [128,2]->[2,128]
                    # transpose per block rebuilds both murt rows at once
                    mr = statsb.tile([P, 4, 3], F32, tag="mr")
                    nc.vector.tensor_scalar_mul(mr[:, :, 1], in0=st_ps[:, 0:4],
                                                scalar1=1.0 / DIMK)
                    t2T = stg.tile([P, 4], F32, tag="t2")
                    nc.vector.tensor_mul(t2T, mr[:, :, 1], mr[:, :, 1])
                    lvT = stg.tile([P, 4], F32, tag="lv")
                    nc.vector.scalar_tensor_tensor(out=lvT, in0=st_ps[:, 4:8],
                                                   scalar=1.0 / DIMK, in1=t2T,
                                                   op0=AluOpType.mult,
                                                   op1=AluOpType.subtract)
                    nc.scalar.activation(out=lvT, in_=lvT, func=AF.Ln, bias=eps128)
                    nc.scalar.activation(out=rsT_t[:, ts(t, 4)], in_=lvT,
                                         func=AF.Exp, scale=-0.5)
                    nc.vector.tensor_scalar_mul(scT_t[:, ts(t, 4)],
                                                in0=rsT_t[:, ts(t, 4)],
                                                scalar1=SCALE)
                    nc.scalar.activation(out=mr[:, :, 2], in_=lvT,
                                         func=AF.Exp, scale=0.5)
                    nc.vector.tensor_copy(mr[:, :, 0], rsT_t[:, ts(t, 4)])
                    # transpose token-major columns back to row-major via PE
                    murt = statsb.tile([3, 512], BF16, tag="mur")
                    row_ps = stps.tile([3, 512], F32, tag="row")
                    for c in range(4):
                        nc.tensor.transpose(row_ps[0:3, ts(c, P)],
                                            in_=mr[:, c, :],
                                            identity=idn_t)
                    nc.vector.tensor_copy(murt, row_ps)
                    # rstd broadcast across partitions (Pool) for q eviction
                    rb_t = rbp.tile([P, 512], BF16, tag="rb")
                    nc.gpsimd.partition_broadcast(rb_t, murt[0:1, :])
                    return xk, murt, rb_t

                def emit_tile_qkv(t, xk, murt, rb_t):
                    # q/k matmuls (feature-major) + LN-fold correction
                    qk_ps = qkps.tile([P, 2, 512], F32, tag="qk")
                    for c in range(2):
                        for k in range(KC):
                            nc.tensor.matmul(qk_ps[:, c, :],
                                             lhsT=w_t[:, k, ts(c, P)],
                                             rhs=xk[:, k, :],
                                             start=(k == 0), stop=False)
                        nc.tensor.matmul(qk_ps[:, c, :],
                                         lhsT=gb_t[:, ts(c, P)],
                                         rhs=murt,
                                         start=False, stop=True)
                    # evict: q scaled by rstd (broadcast row); k unscaled
                    nc.vector.tensor_mul(qn[:, ts(t, 512)], qk_ps[:, 0, :], rb_t)
                    nc.scalar.copy(kn[:, ts(t, 512)], qk_ps[:, 1, :])

                    # v matmuls (token-major) + correction + rstd evict
                    v_ps = vps.tile([P, 4, P], F32, tag="v")
                    for blk in range(4):
                        for k in range(KC):
                            nc.tensor.matmul(v_ps[:, blk, :],
                                             lhsT=xk[:, k, ts(blk, P)],
                                             rhs=w_t[:, k, 256:384],
                                             start=(k == 0), stop=False,
                                             skip_group_check=True)
                        nc.tensor.matmul(v_ps[:, blk, :],
                                         lhsT=murt[:, ts(blk, P)],
                                         rhs=gb_t[:, 256:384],
                                         start=False, stop=True,
                                         skip_group_check=True)
                    for blk in range(4):
                        gblk = 4 * t + blk
                        nc.vector.tensor_scalar_mul(
                            vtok[:, gblk, 0:64],
                            in0=v_ps[:, blk, 0:D],
                            scalar1=rsT_t[:, gblk : gblk + 1])
                        nc.scalar.mul(vtok[:, gblk, 65 : 65 + 64],
                                      in_=v_ps[:, blk, D : 2 * D],
                                      mul=rsT_t[:, gblk : gblk + 1])

                    # RoPE in place on q, k (PE rotate-half permutation)
                    cs = ts(t % 4, 512)
                    for ci, src in enumerate((qn, kn)):
                        sl = src[:, ts(t, 512)]
                        rp_ps = rpps.tile([P, 512], F32, tag="rp")
                        nc.tensor.matmul(rp_ps, lhsT=perm_t, rhs=sl,
                                         start=True, stop=True)
                        ra = stg.tile([P, 512], BF16, tag="ra")
                        nc.vector.tensor_mul(ra, sl, cc_t[:, cs])
                        rb2_ = stg.tile([P, 512], BF16, tag="rb2")
                        nc.vector.tensor_mul(rb2_, rp_ps, ss_t[:, cs])
                        nc.vector.tensor_add(sl, ra, rb2_)

                # software pipeline: stats(t+1) ahead of qkv(t); phase0 PE work
                # (already emitted) only blocks the first correction matmul
                pend = emit_tile_stats(0)
                for t in range(TT):
                    nxt = emit_tile_stats(t + 1) if t + 1 < TT else None
                    emit_tile_qkv(t, *pend)
                    pend = nxt

            # ---------- phase D: attention per (batch, head) + out-proj ------
            with tc.tile_pool(name="pp", bufs=2) as pp, \
                 tc.tile_pool(name="dsb", bufs=2) as dsb, \
                 tc.tile_pool(name="stp", bufs=2, space="PSUM") as stp, \
                 tc.tile_pool(name="avp", bufs=2, space="PSUM") as avp, \
                 tc.tile_pool(name="opp", bufs=2, space="PSUM") as opp, \
                 tc.tile_pool(name="oute", bufs=6) as outep:

                nc.sync.dma_start(out=tri_t, in_=tri_h[:, :])
                nc.sync.dma_start(out=wo_t, in_=wo_h[:, :])
                nc.sync.dma_start(out=onesr_t, in_=onesr_h[:, :])
                nc.sync.dma_start(
                    out=vtok[:, :, :].rearrange("p b (h o) -> p (b h) o", o=65)[:, :, 64:65],
                    in_=onesb_h[:, :].rearrange("p (a o) -> p a o", o=1))

                def emit_scores_J(pair, J, pJ):
                    b, h = pair // 2, pair % 2
                    base = NT * b
                    hr = D * h
                    m = J % 4
                    i0 = 512 * (J // 4)
                    ilen = NT - i0
                    lhs = kn[hr : hr + D, base + P * J : base + P * (J + 1)]
                    scl = scT_t[:, 16 * b + J : 16 * b + J + 1]
                    for c0 in range(0, ilen, 1024):
                        clen = min(1024, ilen - c0)
                        st = stp.tile([P, 1024], F32, tag="st")
                        off0 = P * m if c0 == 0 else 0
                        starts = [off0] if off0 else []
                        starts += list(range(512 if off0 else 0, clen, 512))
                        for boff in starts:
                            n = min(512 - (boff % 512), clen - boff)
                            nc.tensor.matmul(
                                st[:, boff : boff + n],
                                lhsT=lhs,
                                rhs=qn[hr : hr + D,
                                       base + i0 + c0 + boff :
                                       base + i0 + c0 + boff + n],
                                start=True, stop=True, skip_group_check=True)
                        nc.scalar.activation(out=pJ[:, c0 + off0 : c0 + clen],
                                             in_=st[:, off0:clen],
                                             func=AF.Exp, scale=scl)
                    # causal mask on the diagonal 128x128 sub-block
                    eng = nc.vector if J % 2 == 0 else nc.gpsimd
                    eng.tensor_mul(pJ[:, P * m : P * (m + 1)],
                                   pJ[:, P * m : P * (m + 1)], tri_t)

                def emit_av_I(pair, ptiles, I, op_b):
                    b, h = pair // 2, pair % 2
                    base = NT * b
                    hr = D * h
                    av = avp.tile([P, 512], F32, tag="av")
                    last = 4 * I + 3
                    for J in range(0, last + 1):
                        i0 = 512 * (J // 4)
                        cbase = 512 * I - i0
                        off = P * (J % 4) if J // 4 == I else 0
                        nc.tensor.matmul(
                            av[0 : D + 1, off:512],
                            lhsT=vtok[:, 16 * b + J, 65 * h : 65 * h + 65],
                            rhs=ptiles[J][:, cbase + off : cbase + 512],
                            start=(J == 0), stop=(J == last),
                            skip_group_check=True)
                    rec = dsb.tile([1, 512], F32, tag="rec")
                    with nc.allow_low_precision(reason="softmax denom, 2e-2 budget"):
                        nc.vector.reciprocal(rec, av[D : D + 1, 0:512])
                    rcb = dsb.tile([D, 512], F32, tag="rcb")
                    nc.gpsimd.partition_broadcast(rcb, rec)
                    nc.vector.tensor_mul(
                        ohT[hr : hr + D, base + 512 * I : base + 512 * (I + 1)],
                        av[0:D, 0:512], rcb)
                    if op_b is not None:
                        if I > 0:
                            emit_outproj_chunks(op_b, 4 * (I - 1), 4 * I)
                        if I == 3:
                            emit_outproj_chunks(op_b, 12, 16, drain=(op_b == 1))

                def emit_outproj_chunks(b, lo, hi, drain=False):
                    if drain:
                        rot = [nc.vector, nc.scalar, nc.vector, nc.scalar]
                        for t in range(16 * b + lo, 16 * b + hi):
                            ev = outep.tile([P, DIMK], BF16, tag="evd")
                            for cb in range(2):
                                op_ps = opp.tile([P, 512], F32, tag="op")
                                nc.tensor.matmul(op_ps,
                                                 lhsT=ohT[:, ts(t, P)],
                                                 rhs=wo_t[:, ts(cb, 512)],
                                                 start=True, stop=True)
                                eng = rot[(2 * t + cb) % 4]
                                if eng is nc.scalar:
                                    eng.copy(ev[:, ts(cb, 512)], op_ps)
                                else:
                                    eng.tensor_copy(ev[:, ts(cb, 512)], op_ps)
                            nc.sync.dma_start(out=out_h[ts(t, P), :], in_=ev)
                        return
                    rot = [nc.vector, nc.vector, nc.vector, nc.scalar]
                    for tp in range(8 * b + lo // 2, 8 * b + hi // 2):
                        ev = outep.tile([P, 2, DIMK], BF16, tag="ev")
                        for half in range(2):
                            t = 2 * tp + half
                            for cb in range(2):
                                op_ps = opp.tile([P, 512], F32, tag="op")
                                nc.tensor.matmul(op_ps,
                                                 lhsT=ohT[:, ts(t, P)],
                                                 rhs=wo_t[:, ts(cb, 512)],
                                                 start=True, stop=True)
                                eng = rot[(2 * t + cb) % 4]
                                if eng is nc.scalar:
                                    eng.copy(ev[:, half, ts(cb, 512)], op_ps)
                                else:
                                    eng.tensor_copy(ev[:, half, ts(cb, 512)], op_ps)
                        nc.sync.dma_start(
                            out=out_h[ts(tp, 2 * P), :].rearrange(
                                "(a p) c -> p a c", p=P),
                            in_=ev)

                # fine-grained pipeline: 4 score-J-groups of pair p interleave
                # with the 4 AV-I-groups (+ out-proj chunks) of pair p-1
                prev = None
                for pair in range(4):
                    ptl = [pp.tile([P, NT - 512 * (J // 4)], BF16,
                                   tag=f"p{J}", name=f"p{J}_{pair}")
                           for J in range(16)]
                    for g in range(4):
                        for J in range(4 * g, 4 * g + 4):
                            emit_scores_J(pair, J, ptl[J])
                        if prev is not None:
                            pp_, ptl_ = prev
                            emit_av_I(pp_, ptl_, g,
                                      pp_ // 2 if pp_ % 2 == 1 else None)
                    prev = (pair, ptl)
                pp_, ptl_ = prev
                for g in range(4):
                    emit_av_I(pp_, ptl_, g, pp_ // 2)

    nc.finalize()
    return nc
